# revision 1
# baseline (speedup 1.0000x reference)
"""Trainium2 Bass kernel for CausalSelfAttention (no causal mask in reference).

Problem shapes: x [B=2, T=2048, C=1024], H=16 heads, D=64 head dim.
  q/k/v = x @ W{q,k,v}.T ; att = softmax(q k^T / sqrt(D)) ; y = att v
  out = y @ Wp.T + bp

Sharding over 8 NeuronCores: 4 head-groups (4 heads = 256 dims each) x 2
batches.  Core (g, b) computes a partial output for x[b] restricted to head
group g; the host sums the 4 head-group partials per batch and adds bp.

Per-core device program (matmul operands bf16, fp32 PSUM accumulate):
  1. QT = (Wq_g*scale) @ x^T   [256, T]   (d on partitions, t on free axis)
     KT = Wk_g @ x^T           [256, T]
     V  = x @ Wv_g^T           [T, 256]   (natural layout, + ones columns)
  2. per head h, per 512-wide t-chunk:
       S_T[s, t] = KT_h-tile @ QT_h       (scores transposed: s on partitions;
                                           two heads packed in PE row groups)
       P = exp(S_T)                       (no max subtraction: scores are O(1)
                                           by construction, exp is safe)
       Yaug^T = [V_h | 1...1]^T @ P       -> rows 0..63 unnormalized Y^T,
                                             rows 64..127 = softmax denom
                                             (replicated by the ones columns)
       Y^T = Yaug^T[0:64] * recip(rows 64..127)
  3. out_partial = Y^T-tiles^T @ Wp_g^T   [T, 1024]
All layouts chain with zero on-chip transposes.  Emission order interleaves
phase 1 with attention so the Scalar engine (exp, the throughput floor)
starts early and never starves.
"""

import numpy as np
import ml_dtypes

import concourse.bass as bass
import concourse.tile as tile
from concourse import mybir
from concourse.bacc import Bacc
from concourse.bass_utils import run_bass_kernel_spmd

BF16 = mybir.dt.bfloat16
F32 = mybir.dt.float32
NP_BF16 = ml_dtypes.bfloat16

P = 128
C = 1024
H = 16
D = 64
N_CORES = 8
N_GROUPS = 4              # head groups (tensor parallel)
N_BATCH = 2               # data parallel over B
HL = H // N_GROUPS        # 4 local heads
DL = HL * D               # 256 local head dims
CHUNK = 512               # t-chunk width (one PSUM bank of fp32)


def build_program(T: int = 2048) -> bass.Bass:
    KO = C // P            # k-tiles over the C contraction
    TT = T // P            # s/t tiles of 128
    NCH = T // CHUNK       # t-chunks
    KP = DL // P           # k-tiles over local head dims (2)

    nc = Bacc()
    xT_d = nc.declare_dram_parameter("xT", [C, T], BF16, isOutput=False)
    wqT_d = nc.declare_dram_parameter("wqT", [C, DL], BF16, isOutput=False)
    wkT_d = nc.declare_dram_parameter("wkT", [C, DL], BF16, isOutput=False)
    wvT_d = nc.declare_dram_parameter("wvT", [C, DL], BF16, isOutput=False)
    wpT_d = nc.declare_dram_parameter("wpT", [DL, C], BF16, isOutput=False)
    out_d = nc.declare_dram_parameter("out", [T, C], F32, isOutput=True)

    EXP = mybir.ActivationFunctionType.Exp

    with tile.TileContext(nc) as tc:
        with (
            tc.tile_pool(name="const", bufs=1) as cp,
            tc.tile_pool(name="att_s", bufs=2, space="PSUM") as att_s,
            tc.tile_pool(name="accy", bufs=2, space="PSUM") as accy,
            tc.tile_pool(name="accps", bufs=2, space="PSUM") as accps,
            tc.tile_pool(name="expp", bufs=40) as exp_pool,
            tc.tile_pool(name="normp", bufs=4) as norm_pool,
            tc.tile_pool(name="outp", bufs=4) as out_pool,
        ):
            xT_sb = cp.tile([P, KO, T], BF16)
            wqT_sb = cp.tile([P, KO, DL], BF16)
            wkT_sb = cp.tile([P, KO, DL], BF16)
            wvT_sb = cp.tile([P, KO, DL], BF16)
            wpT_sb = cp.tile([P, KP, C], BF16)
            QT_sb = cp.tile([P, KP, T], BF16)
            KT_sb = cp.tile([P, KP, T], BF16)
            # per head: 64 V columns then 64 ones columns; the ones columns
            # make the PV matmul emit the softmax denominator replicated
            # across PSUM partitions 64..127 (partition broadcast for free).
            Vaug_sb = cp.tile([P, TT, HL * 2 * D], BF16)
            YT_sb = cp.tile([P, KP, T], BF16)

            # dummy matmuls on a memset tile fill the DMA lead-in so the
            # PE clock ramp (HAM) is already warm when real matmuls arrive
            warm_sb = cp.tile([P, CHUNK], BF16)
            nc.vector.memset(warm_sb, 0.0)
            for _w in range(2):
                ps_w = accps.tile([P, CHUNK], F32, tag="acc", name="ps_w")
                nc.tensor.matmul(
                    ps_w, lhsT=warm_sb[:, 0:P], rhs=warm_sb, start=True, stop=True
                )

            # DMAs ordered by first use: K weights, x, Q/V weights, Wp last
            wkT_r = wkT_d[:, :].rearrange("(ko p) d -> p ko d", p=P)
            nc.gpsimd.dma_start(out=wkT_sb[:, 0:4, :], in_=wkT_r[:, 0:4, :])
            nc.gpsimd.dma_start(out=wkT_sb[:, 4:8, :], in_=wkT_r[:, 4:8, :])
            # x slabs ordered chunk-major so the first projection group's
            # k-loop unblocks after 1/NCH of the x transfer; slabs spread
            # over two engines' DMA queues so transfers run concurrently
            xT_r = xT_d[:, :].rearrange("(ko p) t -> ko p t", p=P)
            dma_engs = [nc.sync, nc.gpsimd]
            for ch in range(NCH):
                for k in range(KO):
                    dma_engs[ch % 2].dma_start(
                        out=xT_sb[:, k, ch * CHUNK : (ch + 1) * CHUNK],
                        in_=xT_r[k][:, ch * CHUNK : (ch + 1) * CHUNK],
                    )
            for w_d, w_sb in ((wqT_d, wqT_sb), (wvT_d, wvT_sb)):
                nc.sync.dma_start(
                    out=w_sb[:, :, :],
                    in_=w_d[:, :].rearrange("(ko p) d -> p ko d", p=P),
                )
            nc.sync.dma_start(
                out=wpT_sb[:, :, :],
                in_=wpT_d[:, :].rearrange("(kp p) n -> p kp n", p=P),
            )

            vview = Vaug_sb.rearrange("p tt (h e) -> p tt h e", e=2 * D)
            nc.gpsimd.memset(vview[:, :, :, D : 2 * D], 1.0)

            # ---------- emission helpers ----------
            def emit_qk_group(w_sb, o_sb, m, ch):
                ps = accps.tile([P, CHUNK], F32, tag="acc", name="ps")
                for k in range(KO):
                    nc.tensor.matmul(
                        ps,
                        lhsT=w_sb[:, k, m * P : (m + 1) * P],
                        rhs=xT_sb[:, k, ch * CHUNK : (ch + 1) * CHUNK],
                        start=(k == 0),
                        stop=(k == KO - 1),
                    )
                nc.vector.tensor_copy(
                    out=o_sb[:, m, ch * CHUNK : (ch + 1) * CHUNK], in_=ps
                )

            def emit_v_group(m):
                ps = accps.tile([P, CHUNK], F32, tag="acc", name="ps")
                for k in range(KO):
                    nc.tensor.matmul(
                        ps[:, 0:DL],
                        lhsT=xT_sb[:, k, m * P : (m + 1) * P],
                        rhs=wvT_sb[:, k, :],
                        start=(k == 0),
                        stop=(k == KO - 1),
                    )
                nc.vector.tensor_copy(
                    out=vview[:, m, :, 0:D],
                    in_=ps[:, 0:DL].rearrange("p (h e) -> p h e", e=D),
                )

            exps = {}  # (ch, hp) -> list of exp tiles

            def emit_sexp(ch, hp):
                t0 = ch * CHUNK
                lst = []
                for s in range(TT):
                    ps_s = att_s.tile([P, 2 * CHUNK], F32, tag="s", name="ps_s")
                    # two heads packed into PE row groups (K=64 each)
                    nc.tensor.matmul(
                        ps_s[:, 0:CHUNK],
                        lhsT=KT_sb[0:64, hp, s * P : (s + 1) * P],
                        rhs=QT_sb[0:64, hp, t0 : t0 + CHUNK],
                        start=True,
                        stop=True,
                    )
                    nc.tensor.matmul(
                        ps_s[:, CHUNK : 2 * CHUNK],
                        lhsT=KT_sb[64:128, hp, s * P : (s + 1) * P],
                        rhs=QT_sb[64:128, hp, t0 : t0 + CHUNK],
                        start=True,
                        stop=True,
                    )
                    ex = exp_pool.tile([P, 2 * CHUNK], BF16, tag="e", name="ex")
                    nc.scalar.activation(out=ex, in_=ps_s, func=EXP)
                    lst.append(ex)
                exps[(ch, hp)] = lst

            def emit_pv(ch, hp):
                t0 = ch * CHUNK
                lst = exps.pop((ch, hp))
                ps_y = {}
                for ha in range(2):
                    ps_y[ha] = accy.tile([P, CHUNK], F32, tag="y", name="ps_y")
                for ha in range(2):
                    h = hp * 2 + ha
                    for s in range(TT):
                        nc.tensor.matmul(
                            ps_y[ha],
                            lhsT=Vaug_sb[:, s, h * 2 * D : (h + 1) * 2 * D],
                            rhs=lst[s][:, ha * CHUNK : (ha + 1) * CHUNK],
                            start=(s == 0),
                            stop=(s == TT - 1),
                        )
                    recip = norm_pool.tile([D, CHUNK], F32, tag="r", name="recip")
                    nc.vector.reciprocal(out=recip, in_=ps_y[ha][D : 2 * D, :])
                    nc.vector.tensor_mul(
                        out=YT_sb[ha * D : (ha + 1) * D, hp, t0 : t0 + CHUNK],
                        in0=ps_y[ha][0:D, :],
                        in1=recip,
                    )

            def emit_outproj(ch, last=False):
                for mt in range(CHUNK // P):
                    m = ch * (CHUNK // P) + mt
                    for n2 in range(C // CHUNK):
                        ps_o = accps.tile([P, CHUNK], F32, tag="acc", name="ps_o")
                        for kk in range(KP):
                            nc.tensor.matmul(
                                ps_o,
                                lhsT=YT_sb[:, kk, m * P : (m + 1) * P],
                                rhs=wpT_sb[:, kk, n2 * CHUNK : (n2 + 1) * CHUNK],
                                start=(kk == 0),
                                stop=(kk == KP - 1),
                            )
                        o_sb = out_pool.tile([P, CHUNK], F32, tag="o", name="o_sb")
                        # in the tail the exp stream is done, so the Scalar
                        # engine is free to take half the drain copies
                        if last and n2 % 2 == 0:
                            nc.scalar.copy(out=o_sb, in_=ps_o)
                        else:
                            nc.vector.tensor_copy(out=o_sb, in_=ps_o)
                        dma_engs[n2 % 2].dma_start(
                            out=out_d[
                                m * P : (m + 1) * P,
                                n2 * CHUNK : (n2 + 1) * CHUNK,
                            ],
                            in_=o_sb,
                        )

            # ---------- emission order ----------
            # scores+exp for head-pair hp needs only K tile hp (all chunks)
            # and Q tile hp (that chunk), so the Scalar engine (the
            # throughput floor) starts exp'ing ~15us in; V and the remaining
            # Q chunks fill PE time under those exps, then a lookahead-1
            # software pipeline keeps ACT fed through the PV/proj phases.
            for ch in range(NCH):
                emit_qk_group(wkT_sb, KT_sb, 0, ch)
            emit_qk_group(wqT_sb, QT_sb, 0, 0)
            emit_sexp(0, 0)
            for ch in range(NCH):
                emit_qk_group(wkT_sb, KT_sb, 1, ch)
            emit_qk_group(wqT_sb, QT_sb, 1, 0)
            emit_sexp(0, 1)
            # V groups interleaved with the remaining Q groups: the short
            # (N=256) V matmuls then have long Q matmuls to hide their
            # per-matmul weight loads under (PE load-ahead queue)
            vq = []
            for m in range(TT // 2):
                vq.append(("v", m))
            if NCH > 1:
                vq.append(("q", (0, 1)))
                vq.append(("sexp", (1, 0)))
            for m in range(TT // 2, TT):
                vq.append(("v", m))
            if NCH > 1:
                vq.append(("q", (1, 1)))
                vq.append(("sexp", (1, 1)))
            qrest = [(m, ch) for ch in range(2, NCH) for m in range(KP)]
            mixed = []
            vi = 0
            for item in vq:
                mixed.append(item)
                if item[0] == "v":
                    vi += 1
                    if vi % 3 == 0 and qrest:
                        mixed.append(("q", qrest.pop(0)))
            for kind, arg in mixed:
                if kind == "v":
                    emit_v_group(arg)
                elif kind == "q":
                    emit_qk_group(wqT_sb, QT_sb, arg[0], arg[1])
                else:
                    emit_sexp(arg[0], arg[1])
            for m, ch in qrest:
                emit_qk_group(wqT_sb, QT_sb, m, ch)
            # output projection deferred by one chunk: it becomes PE filler
            # work for the stretches where PV is paced by the exp drain
            for ch in range(NCH):
                if 2 <= ch + 1 < NCH:
                    emit_sexp(ch + 1, 0)
                emit_pv(ch, 0)
                if ch >= 1:
                    emit_outproj(ch - 1)
                emit_pv(ch, 1)
                if 2 <= ch + 1 < NCH:
                    emit_sexp(ch + 1, 1)
            emit_outproj(NCH - 1, last=True)
    # run the Bacc passes (matmul-wait relocation, wait splitting, reg alloc)
    nc.finalize()
    return nc


def shard_inputs(x, Wk, Wq, Wv, Wp, T=2048):
    """Build the 8 per-core input dicts (host-side transposes + bf16 casts)."""
    scale = 1.0 / np.sqrt(np.float32(D))
    x = np.asarray(x, np.float32)
    Wk = np.asarray(Wk, np.float32)
    Wq = np.asarray(Wq, np.float32)
    Wv = np.asarray(Wv, np.float32)
    Wp = np.asarray(Wp, np.float32)

    xT = [
        np.ascontiguousarray(x[b, :T].T.astype(NP_BF16)) for b in range(x.shape[0])
    ]
    in_maps = []
    for g in range(N_GROUPS):
        sl = slice(g * DL, (g + 1) * DL)
        wqT = np.ascontiguousarray((Wq[sl] * scale).T.astype(NP_BF16))
        wkT = np.ascontiguousarray(Wk[sl].T.astype(NP_BF16))
        wvT = np.ascontiguousarray(Wv[sl].T.astype(NP_BF16))
        wpT = np.ascontiguousarray(Wp[:, sl].T.astype(NP_BF16))
        for b in range(len(xT)):
            in_maps.append(
                {"xT": xT[b], "wqT": wqT, "wkT": wkT, "wvT": wvT, "wpT": wpT}
            )
    return in_maps


_PROGRAM = None


def kernel(x, Wk, Wq, Wv, Wp, bp):
    global _PROGRAM
    x = np.asarray(x, np.float32)
    bp = np.asarray(bp, np.float32)
    B, T, _ = x.shape

    if _PROGRAM is None:
        _PROGRAM = build_program(T)
    nc = _PROGRAM

    in_maps = shard_inputs(x, Wk, Wq, Wv, Wp, T=T)
    res = run_bass_kernel_spmd(nc, in_maps, core_ids=list(range(N_CORES)))
    parts = [r["out"] for r in res.results]

    out = np.zeros((B, T, C), np.float32)
    for g in range(N_GROUPS):
        for b in range(B):
            out[b] += parts[g * N_BATCH + b]
    out += bp
    return out



# revision 16
# speedup vs baseline: 1.1232x; 1.1232x over previous
"""Trainium2 Bass kernel for CausalSelfAttention (no causal mask in reference).

Problem shapes: x [B=2, T=2048, C=1024], H=16 heads, D=64 head dim.
  q/k/v = x @ W{q,k,v}.T ; att = softmax(q k^T / sqrt(D)) ; y = att v
  out = y @ Wp.T + bp

Sharding over 8 NeuronCores: 4 head-groups (4 heads = 256 dims each) x 2
batches.  Core (g, b) computes a partial output for x[b] restricted to head
group g; the host sums the 4 head-group partials per batch and adds bp.

Precision design: everything is bf16 except the score matmul, which runs
as an fp8e4m3 DoubleRow matmul (2x cheaper per the per-row cost model).
The DoubleRow second sub-row carries the fp8 RESIDUAL of q: with
  QT sub0 = fp8(q), sub1 = fp8(q - fp8(q)),  KT sub0 = sub1 = fp8(k)
the matmul computes (q8 + qr8).k8 = q.k8 exactly up to the k quantization
(~1.1% logit jitter -> ~0.5% output error), at zero extra PE cost since
both sub-rows stream in one DoubleRow pass.  The v/P/out path must stay
bf16: y is an attention-average of v, so elementwise noise on P or V
passes through at full relative strength.

Per-core device program (fp32 PSUM accumulate):
  1. q/k/v = x @ W.T in bf16 (8 k-tiles, N=512);  q/k psum -> fp8 copies
     into the DoubleRow layout [p, m-block, sub, T] (partition 64a+d,
     m-block mb <-> head 2*mb+a); V -> Vone [p, s-tile, per head 64 v
     cols + 1 ones col] bf16 (the ones column makes the PV matmul emit
     the softmax denominator in psum column 64 for free).
  2. per (t-chunk, head pair), per s-tile:
       S_raw[s,t] = q.k8 = 8 * (q.k/sqrt(D))   (fp8 DoubleRow, 256cyc)
       P = exp(S_raw/8) -> bf16, split across TWO engines:
         ACT: real exp (scale=1/8)
         DVE: Schraudolph bit-exp: i16 = round(S_raw*a + b) viewed as
           bf16 == 2^(S/8*log2e) to ~0.5% accuracy
     per head, per 128-wide t-tile: PV streams N=65 (64 v cols + ones):
       Yaug[t, 0:65] = sum_s P^T-slice.T @ Vone_h   (16 matmuls, 65cyc)
       Ynat[t, d] = Yaug[:, 0:64] * recip(Yaug[:, 64])   (per-partition
         scalar broadcast: DVE reciprocal + tensor_scalar multiply)
  3. Ynat [t, d] -> Y^T via XBAR dma_start_transpose (DMA engines, zero
     compute-engine time); out_partial = Y^T^T @ Wp^T in bf16, staged
     bf16 -> DMA (host sums 4 head-group partials per batch, adds bp).
"""

import numpy as np
import ml_dtypes

import concourse.bass as bass
import concourse.tile as tile
from concourse import mybir
from concourse.bacc import Bacc
from concourse.bass_utils import run_bass_kernel_spmd

BF16 = mybir.dt.bfloat16
F32 = mybir.dt.float32
FP8 = mybir.dt.float8e4
I16 = mybir.dt.int16
NP_BF16 = ml_dtypes.bfloat16

P = 128
C = 1024
H = 16
D = 64
N_CORES = 8
N_GROUPS = 4              # head groups (tensor parallel)
N_BATCH = 2               # data parallel over B
HL = H // N_GROUPS        # 4 local heads
DL = HL * D               # 256 local head dims
CHUNK = 512               # t-chunk width
DR = mybir.MatmulPerfMode.DoubleRow

# weighted round-robin for the exp engine split (ACT, DVE)
EXP_WEIGHTS = (79.0, 49.0)
# Schraudolph exp to bf16: scores psum holds 8*s; bf16 bits =
# 128*(s*log2e + 127) - 5.5 (the -5.5 centers the mantissa-chord error)
A_SCH = 128.0 * 1.4426950408889634 / 8.0
B_SCH = 127.0 * 128.0 - 5.5


def build_program(T: int = 2048) -> bass.Bass:
    KO = C // P            # 8 k-tiles over the C contraction
    TT = T // P            # 16 s/t tiles of 128
    NCH = T // CHUNK       # 4 t-chunks
    KP = DL // P           # 2 k-tiles over local head dims

    nc = Bacc()
    xTb_d = nc.declare_dram_parameter("xTb", [C, T], BF16, isOutput=False)
    wqT_d = nc.declare_dram_parameter("wqTb", [C, DL], BF16, isOutput=False)
    wkT_d = nc.declare_dram_parameter("wkTb", [C, DL], BF16, isOutput=False)
    wvT_d = nc.declare_dram_parameter("wvTb", [C, DL], BF16, isOutput=False)
    wpT_d = nc.declare_dram_parameter("wpT", [DL, C], BF16, isOutput=False)
    out_d = nc.declare_dram_parameter("out", [T, C], BF16, isOutput=True)
    out_r = out_d[:, :].rearrange("(tt p) c -> p tt c", p=P)

    EXP = mybir.ActivationFunctionType.Exp
    MULT = mybir.AluOpType.mult
    ADD = mybir.AluOpType.add
    SUB = mybir.AluOpType.subtract

    with tile.TileContext(nc) as tc:
        with (
            tc.tile_pool(name="const", bufs=1) as cp,
            tc.tile_pool(name="att_s", bufs=2, space="PSUM") as att_s,
            tc.tile_pool(name="accy", bufs=2, space="PSUM") as accy,
            tc.tile_pool(name="accps", bufs=2, space="PSUM") as accps,
            tc.tile_pool(name="expp", bufs=40) as exp_pool,
            tc.tile_pool(name="normp", bufs=4) as norm_pool,
            tc.tile_pool(name="outp", bufs=2) as out_pool,
        ):
            xTb_sb = cp.tile([P, KO, T], BF16)
            wqT_sb = cp.tile([P, KO, DL], BF16)
            wkT_sb = cp.tile([P, KO, DL], BF16)
            wvT_sb = cp.tile([P, KO, DL], BF16)
            wpT_sb = cp.tile([P, KP, C], BF16)
            # q/k DoubleRow layout [p, m-block, sub, T]: partition 64a+d,
            # m-block mb <-> head 2*mb+a dim d.  QT sub0 = fp8(q), sub1 =
            # fp8 residual; KT sub0 = sub1 = fp8(k).
            QT_sb = cp.tile([P, 2, 2, T], FP8)
            KT_sb = cp.tile([P, 2, 2, T], FP8)
            # V natural layout + ones column per head: [p, s-tile, 4*65]
            Vone_sb = cp.tile([P, TT, HL * (D + 1)], BF16)
            Ynat_sb = cp.tile([P, TT, DL], BF16)
            YT_sb = cp.tile([P, KP, T], BF16)

            # dummy matmuls on a memset tile fill the DMA lead-in so the
            # PE clock ramp (HAM) is already warm when real matmuls arrive
            warm_sb = cp.tile([P, CHUNK], BF16)
            nc.vector.memset(warm_sb, 0.0)
            for _w in range(2):
                ps_w = accps.tile([P, CHUNK], F32, tag="acc", name="ps_w")
                nc.tensor.matmul(
                    ps_w, lhsT=warm_sb[:, 0:P], rhs=warm_sb, start=True, stop=True
                )

            # DMAs ordered by first use, all issued from the SP hwdge queue
            nc.sync.dma_start(
                out=wkT_sb[:, :, :],
                in_=wkT_d[:, :].rearrange("(ko p) d -> p ko d", p=P),
            )
            xTb_r = xTb_d[:, :].rearrange("(ko p) t -> p ko t", p=P)
            nc.sync.dma_start(out=xTb_sb[:, :, 0:CHUNK], in_=xTb_r[:, :, 0:CHUNK])
            nc.sync.dma_start(
                out=wqT_sb[:, :, :],
                in_=wqT_d[:, :].rearrange("(ko p) d -> p ko d", p=P),
            )
            for ch in range(1, NCH):
                nc.sync.dma_start(
                    out=xTb_sb[:, :, ch * CHUNK : (ch + 1) * CHUNK],
                    in_=xTb_r[:, :, ch * CHUNK : (ch + 1) * CHUNK],
                )
            nc.sync.dma_start(
                out=wvT_sb[:, :, :],
                in_=wvT_d[:, :].rearrange("(ko p) d -> p ko d", p=P),
            )
            nc.sync.dma_start(
                out=wpT_sb[:, :, :],
                in_=wpT_d[:, :].rearrange("(kp p) n -> p kp n", p=P),
            )

            vview = Vone_sb.rearrange("p s (h e) -> p s h e", e=D + 1)
            nc.gpsimd.memset(vview[:, :, :, D : D + 1], 1.0)

            # ---------- emission helpers ----------
            def emit_qk_group(w_sb, o_sb, m, ch, resid):
                ps = accps.tile([P, CHUNK], F32, tag="acc", name="ps")
                for k in range(KO):
                    nc.tensor.matmul(
                        ps,
                        lhsT=w_sb[:, k, m * P : (m + 1) * P],
                        rhs=xTb_sb[:, k, ch * CHUNK : (ch + 1) * CHUNK],
                        start=(k == 0),
                        stop=(k == KO - 1),
                    )
                sub0 = o_sb[:, m, 0, ch * CHUNK : (ch + 1) * CHUNK]
                sub1 = o_sb[:, m, 1, ch * CHUNK : (ch + 1) * CHUNK]
                nc.vector.tensor_copy(out=sub0, in_=ps)
                if resid:
                    # sub1 = fp8(q - fp8(q)): DoubleRow adds it back in the
                    # score matmul, cancelling the q quantization error
                    nc.vector.tensor_sub(sub1, ps, sub0)
                else:
                    nc.vector.tensor_copy(out=sub1, in_=ps)

            def emit_v_group(m):
                ps = accps.tile([P, CHUNK], F32, tag="acc", name="ps")
                for k in range(KO):
                    nc.tensor.matmul(
                        ps[:, 0:DL],
                        lhsT=xTb_sb[:, k, m * P : (m + 1) * P],
                        rhs=wvT_sb[:, k, :],
                        start=(k == 0),
                        stop=(k == KO - 1),
                    )
                nc.vector.tensor_copy(
                    out=vview[:, m, :, 0:D],
                    in_=ps[:, 0:DL].rearrange("p (h e) -> p h e", e=D),
                )

            # weighted round-robin exp-engine chooser
            eng_credit = [0.0] * len(EXP_WEIGHTS)

            def next_exp_engine():
                wsum = sum(EXP_WEIGHTS)
                for e in range(len(eng_credit)):
                    eng_credit[e] += EXP_WEIGHTS[e] / wsum
                e = max(range(len(eng_credit)), key=lambda i: eng_credit[i])
                eng_credit[e] -= 1.0
                return e

            exps = {}  # (ch, hp) -> list of TT exp tiles [P, 2*CHUNK] bf16

            def emit_sexp(ch, hp):
                t0 = ch * CHUNK
                lst = []
                for s in range(TT):
                    ps_s = att_s.tile([P, 2 * CHUNK], F32, tag="s", name="ps_s")
                    for ha in range(2):
                        h = hp * 2 + ha
                        mb, a = h // 2, h % 2
                        nc.tensor.matmul(
                            ps_s[:, ha * CHUNK : (ha + 1) * CHUNK],
                            lhsT=KT_sb[
                                64 * a : 64 * a + 64, mb, :, s * P : (s + 1) * P
                            ],
                            rhs=QT_sb[64 * a : 64 * a + 64, mb, :, t0 : t0 + CHUNK],
                            start=True,
                            stop=True,
                            perf_mode=DR,
                        )
                    ex = exp_pool.tile([P, 2 * CHUNK], BF16, tag="e", name="ex")
                    if next_exp_engine() == 0:
                        nc.scalar.activation(
                            out=ex, in_=ps_s, func=EXP, scale=1.0 / 8.0
                        )
                    else:
                        nc.vector.tensor_scalar(
                            ex.bitcast(I16), ps_s, A_SCH, B_SCH, MULT, ADD
                        )
                    lst.append(ex)
                exps[(ch, hp)] = lst

            def emit_pv(ch, hp):
                lst = exps.pop((ch, hp))
                for ha in range(2):
                    h = hp * 2 + ha
                    for tt in range(CHUNK // P):
                        m = ch * (CHUNK // P) + tt
                        ps_y = accy.tile([P, CHUNK], F32, tag="y", name="ps_y")
                        for s in range(TT):
                            nc.tensor.matmul(
                                ps_y[:, 0 : D + 1],
                                lhsT=lst[s][
                                    :, ha * CHUNK + tt * P : ha * CHUNK + (tt + 1) * P
                                ],
                                rhs=Vone_sb[:, s, h * (D + 1) : (h + 1) * (D + 1)],
                                start=(s == 0),
                                stop=(s == TT - 1),
                            )
                        recip = norm_pool.tile([P, 1], F32, tag="r", name="recip")
                        nc.vector.reciprocal(out=recip, in_=ps_y[:, D : D + 1])
                        nc.vector.tensor_scalar(
                            Ynat_sb[:, m, h * D : (h + 1) * D],
                            ps_y[:, 0:D],
                            recip[:, 0:1],
                            None,
                            MULT,
                        )

            def emit_xbar(ch):
                # Ynat [t, d] -> YT [d, t] on the DMA XBAR (no engine time)
                for tt in range(CHUNK // P):
                    m = ch * (CHUNK // P) + tt
                    for kk in range(KP):
                        nc.sync.dma_start_transpose(
                            out=YT_sb[:, kk, m * P : (m + 1) * P],
                            in_=Ynat_sb[:, m, kk * P : (kk + 1) * P],
                        )

            def emit_outproj(ch):
                stage = out_pool.tile([P, 2, C], BF16, tag="o", name="o_sb")
                for mt in range(CHUNK // P):
                    m = ch * (CHUNK // P) + mt
                    for n2 in range(C // CHUNK):
                        ps_o = accps.tile([P, CHUNK], F32, tag="acc", name="ps_o")
                        for kk in range(KP):
                            nc.tensor.matmul(
                                ps_o,
                                lhsT=YT_sb[:, kk, m * P : (m + 1) * P],
                                rhs=wpT_sb[:, kk, n2 * CHUNK : (n2 + 1) * CHUNK],
                                start=(kk == 0),
                                stop=(kk == KP - 1),
                            )
                        # psum -> bf16 staging on ACT (GPSIMD can't read
                        # PSUM; DVE is loaded with exp + normalize work)
                        nc.scalar.copy(
                            out=stage[:, mt % 2, n2 * CHUNK : (n2 + 1) * CHUNK],
                            in_=ps_o,
                        )
                    if mt % 2 == 1:
                        nc.sync.dma_start(
                            out=out_r[:, m - 1 : m + 1, :],
                            in_=stage[:, :, :],
                        )
                        if mt == 1:
                            stage = out_pool.tile([P, 2, C], BF16, tag="o",
                                                  name="o_sb")

            # ---------- emission order ----------
            # K (both m-blocks, all chunks) must land before any score; then
            # Q chunk 0 -> scores+exp start ~8us in.  V and remaining Q fill
            # PE time under the ch0/ch1 exps; from there a software pipeline
            # runs pv(ch) BEFORE sexp(ch+1) so the DVE drain of the PV psum
            # ring is never queued behind a fresh batch of DVE exps.
            for ch in range(NCH):
                emit_qk_group(wkT_sb, KT_sb, 0, ch, resid=False)
                emit_qk_group(wkT_sb, KT_sb, 1, ch, resid=False)
            emit_qk_group(wqT_sb, QT_sb, 0, 0, resid=True)
            emit_qk_group(wqT_sb, QT_sb, 1, 0, resid=True)
            emit_sexp(0, 0)
            emit_sexp(0, 1)
            # V groups interleaved with the remaining Q groups
            vq = []
            for m in range(TT // 2):
                vq.append(("v", m))
            if NCH > 1:
                vq.append(("q", (0, 1)))
                vq.append(("q", (1, 1)))
                vq.append(("sexp", (1, 0)))
            for m in range(TT // 2, TT):
                vq.append(("v", m))
            if NCH > 1:
                vq.append(("sexp", (1, 1)))
            qrest = [(m, ch) for ch in range(2, NCH) for m in range(KP)]
            mixed = []
            vi = 0
            for item in vq:
                mixed.append(item)
                if item[0] == "v":
                    vi += 1
                    if vi % 3 == 0 and qrest:
                        mixed.append(("q", qrest.pop(0)))
            for kind, arg in mixed:
                if kind == "v":
                    emit_v_group(arg)
                elif kind == "q":
                    emit_qk_group(wqT_sb, QT_sb, arg[0], arg[1], resid=True)
                else:
                    emit_sexp(arg[0], arg[1])
            for m, ch in qrest:
                emit_qk_group(wqT_sb, QT_sb, m, ch, resid=True)
            # steady state: pv first (DVE drains psum ring immediately),
            # then next chunk's scores/exps, outproj one chunk behind
            for ch in range(NCH):
                emit_pv(ch, 0)
                if 2 <= ch + 1 < NCH:
                    emit_sexp(ch + 1, 0)
                if ch >= 1:
                    emit_outproj(ch - 1)
                emit_pv(ch, 1)
                emit_xbar(ch)
                if 2 <= ch + 1 < NCH:
                    emit_sexp(ch + 1, 1)
            emit_outproj(NCH - 1)
    # run the Bacc passes (matmul-wait relocation, wait splitting, reg alloc)
    nc.finalize()
    return nc


def shard_inputs(x, Wk, Wq, Wv, Wp, T=2048):
    """Build the 8 per-core input dicts (host-side transposes + casts)."""
    x = np.asarray(x, np.float32)
    Wk = np.asarray(Wk, np.float32)
    Wq = np.asarray(Wq, np.float32)
    Wv = np.asarray(Wv, np.float32)
    Wp = np.asarray(Wp, np.float32)

    xTb = [
        np.ascontiguousarray(x[b, :T].T.astype(NP_BF16)) for b in range(x.shape[0])
    ]
    in_maps = []
    for g in range(N_GROUPS):
        sl = slice(g * DL, (g + 1) * DL)
        wqTb = np.ascontiguousarray(Wq[sl].T.astype(NP_BF16))
        wkTb = np.ascontiguousarray(Wk[sl].T.astype(NP_BF16))
        wvTb = np.ascontiguousarray(Wv[sl].T.astype(NP_BF16))
        wpT = np.ascontiguousarray(Wp[:, sl].T.astype(NP_BF16))
        for b in range(len(xTb)):
            in_maps.append(
                {"xTb": xTb[b], "wqTb": wqTb, "wkTb": wkTb, "wvTb": wvTb,
                 "wpT": wpT}
            )
    return in_maps


_PROGRAM = None


def kernel(x, Wk, Wq, Wv, Wp, bp):
    global _PROGRAM
    x = np.asarray(x, np.float32)
    bp = np.asarray(bp, np.float32)
    B, T, _ = x.shape

    if _PROGRAM is None:
        _PROGRAM = build_program(T)
    nc = _PROGRAM

    in_maps = shard_inputs(x, Wk, Wq, Wv, Wp, T=T)
    res = run_bass_kernel_spmd(nc, in_maps, core_ids=list(range(N_CORES)))
    parts = [r["out"] for r in res.results]

    out = np.zeros((B, T, C), np.float32)
    for g in range(N_GROUPS):
        for b in range(B):
            out[b] += np.asarray(parts[g * N_BATCH + b], dtype=np.float32)
    out += bp
    return out
